# revision 11
# baseline (speedup 1.0000x reference)
"""Paged causal GQA prefill attention on 8 TRN2 NeuronCores.

Problem: B=4 seqs x S=1024 tokens, HQ=32 query heads, HK=8 KV heads, D=128,
paged KV cache (16 blocks x 256), causal, softmax scale 1/sqrt(128).

Sharding: tensor-parallel over heads. Core c owns KV head c and the G=4
query heads [4c, 4c+4) for all 4 sequences -> 16 (seq, head) units per core,
perfectly balanced, no collectives (output is disjoint across cores).

Per-unit algorithm (S^T layout, bf16 matmuls, f32 accumulation):
  S^T[k,q] = K^T.T @ Q^T   (lhsT = K^T[d,k] tile, rhs = Q^T[d,q], PSUM f32)
  P^T[k,q] = exp(SCALE * S^T)  split across TWO engines:
     chunks 0,1,2,3,6 (3584 cols): ScalarE ACTIVATE exp (exact)
     chunks 4,5,7 (1024 cols): DVE Schraudolph exp-approx
       bf16(exp(x*SCALE)) ~= bitcast_bf16(int16(x*EXP_A + EXP_B))
  diag blocks: zero k>q half           (GPSIMD affine_select)
  O[q, 0:129] = sum_j P^T_j.T @ [V_j | 1]  (PSUM accumulate over k chunks;
               col 128 is the softmax denominator, no separate reduction)
  host divides numerator by denominator column.

Why the split: ScalarE ACTIVATE costs (N+352)/1.2 ns; all-exp-on-ScalarE is
74us vs ~66us of PE matmul -> ScalarE was the trace bottleneck (82% busy).
Moving 1536 cols to DVE rebalances: ScalarE ~55us, DVE ~58us (incl. the
o-PSUM casts), PE ~66-70us critical. The Schraudolph common-mode error
cancels in the softmax normalization (denominator sums the same approx
values); the residual ~2% rms per-weight ripple lands well under the 2e-2
gate (measured).

Score PSUM groups (3-bank tiles, 2-slot rotation as in the 91us baseline —
a 4-tile rotation measured slower from tighter producer-consumer slack).
ScalarE and DVE slices of one group tile never share a PSUM bank
(concurrent ScE/DVE reads of one bank are not allowed):
  G0 = [c0 sc 0:1024            | c4 dve 1024:1536]      (banks 0-1 | 2)
  G1 = [c2 sc 0:768 c6 768:1024 | c5,c7 dve 1024:1536]   (banks 0-1 | 2)
  G2 = [c1 sc 0:896 c3 896:1536]
DVE queue order per head: TS(c4), cast p0, TS(c5c7), cast p1, cast p2 —
every cast is enqueued immediately after its PV pack so o-PSUM recycling
never waits behind exp work.

Output DRAM layout is [B, G, 128, NT, D+1] (partition-major, identical to
the SBUF ob tile) so the store is one fully-contiguous 2064B-per-partition
DMA per head; the host re-permutes. DMA triggers are merged (the SP queue
was 70% busy on 661ns-per-call descriptor generation in the baseline).
"""

import numpy as np
import ml_dtypes
import math as _math
from contextlib import ExitStack

import concourse.bass as bass
import concourse.tile as tile
from concourse import bacc, mybir
from concourse.bass_utils import run_bass_kernel_spmd

B, S, HQ, HK, D = 4, 1024, 32, 8, 128
BS = 256
G = HQ // HK            # 4 query heads per KV head
NCORES = 8
NT = S // 128           # 8 key chunks / query tiles of 128
SCALE = 1.0 / float(np.sqrt(D))

BF16 = mybir.dt.bfloat16
F32 = mybir.dt.float32
I16 = mybir.dt.int16
_BF16_NP = ml_dtypes.bfloat16

EXP_A = SCALE * 128.0 / _math.log(2.0)  # fold softmax scale into the fma
EXP_B = 128.0 * 127.0 - 6.0             # bias, c=6 calibrated for min rms

# chunk j covers keys [128j, 128(j+1)) and queries q in [128j, S)
CHUNK_W = {j: S - 128 * j for j in range(NT)}

# ScalarE chunks packed in the pt tile (bf16 SBUF), group-major
SC_OFF = {0: 0, 2: 1024, 6: 1792, 1: 2048, 3: 2944}
PT_COLS = 3584
# DVE chunks: d0 tile holds chunk 4 (512), d1 tile holds chunks 5+7 (512)
DVE_TILE = {4: "d0", 5: "d1", 7: "d1"}
DVE_OFF = {4: 0, 5: 0, 7: 384}

# score psum groups: ("sc"/"dve" chunk lists with psum col offsets).
# DVE slices start on a PSUM bank boundary (col 1024 = bank 2 of the
# 3-bank group tile) so ScalarE and DVE never read the same bank.
# Chunk 0 stays on ScalarE: it is the first accumulation step of every
# PV output tile, and routing it through DVE measured -11us.
GROUPS = [
    {"sc": [(0, 0)], "dve": [(4, 1024)]},
    {"sc": [(2, 0), (6, 768)], "dve": [(5, 1024), (7, 1408)]},
    {"sc": [(1, 0), (3, 896)], "dve": []},
]
SLOT_W = 1536

_NC_CACHE = None


def _emit(tc, qT, kT, vp, out):
    nc = tc.nc
    Exp = mybir.ActivationFunctionType.Exp

    with ExitStack() as ctx:
        kv_pool = ctx.enter_context(tc.tile_pool(name="kv", bufs=3))
        q_pool = ctx.enter_context(tc.tile_pool(name="q", bufs=4))
        pt_pool = ctx.enter_context(tc.tile_pool(name="pt", bufs=3))
        dve_pool = ctx.enter_context(tc.tile_pool(name="dve", bufs=3))
        s_psum = ctx.enter_context(tc.tile_pool(name="s_psum", bufs=2, space="PSUM"))
        o_psum = ctx.enter_context(tc.tile_pool(name="o_psum", bufs=2, space="PSUM"))
        ob_pool = ctx.enter_context(tc.tile_pool(name="ob", bufs=6))
        singles = ctx.enter_context(tc.tile_pool(name="singles", bufs=1))

        # trigger the exp ACT_TABLE_LOAD (~2.7us) during the initial DMAs
        warm = singles.tile([1, 1], F32)
        nc.vector.memset(warm, 0.0)
        nc.scalar.activation(out=warm, in_=warm, func=Exp)

        heads = [(b, l) for b in range(B) for l in range(G)]
        stage = {}
        kv_cur = None

        def load_kv(bb):
            kt_t = kv_pool.tile([D, S], BF16, tag="kt")
            nc.sync.dma_start(out=kt_t[:, :128], in_=kT[bb][:, :128])
            nc.sync.dma_start(out=kt_t[:, 128:], in_=kT[bb][:, 128:])
            vp_t = kv_pool.tile([128, NT, D + 1], BF16, tag="vp")
            nc.sync.dma_start(out=vp_t, in_=vp[bb])
            return kt_t, vp_t

        # Software pipeline staggered by one head: PE runs QK^T(n) while
        # ScalarE/DVE exp head n-1..n scores; PV(n-1) P^T is ready by then.
        for n in range(len(heads) + 1):
            if n < len(heads):
                b, l = heads[n]
                if n == 0:
                    kt0 = kv_pool.tile([D, S], BF16, tag="kt")
                    nc.sync.dma_start(out=kt0[:, :128], in_=kT[0][:, :128])
                q_t = q_pool.tile([D, S], BF16, tag="q")
                if n == 0:
                    nc.sync.dma_start(out=q_t[:, :512], in_=qT[b, l][:, :512])
                    nc.sync.dma_start(out=q_t[:, 512:], in_=qT[b, l][:, 512:])
                    nc.sync.dma_start(out=kt0[:, 128:], in_=kT[0][:, 128:])
                    vp0 = kv_pool.tile([128, NT, D + 1], BF16, tag="vp")
                    nc.sync.dma_start(out=vp0, in_=vp[0])
                    kv_next = (kt0, vp0)
                else:
                    nc.sync.dma_start(out=q_t, in_=qT[b, l])
                if l == 0:
                    kv_cur, kv_next = kv_next, None
                if l == G - 1 and b + 1 < B:
                    # prefetch the next sequence's K/V one head early
                    kv_next = load_kv(b + 1)
                kt_t, vp_t = kv_cur

                pt_t = pt_pool.tile([128, PT_COLS], BF16, tag="pt")
                d0_t = dve_pool.tile([128, 512], I16, tag="d0")
                d1_t = dve_pool.tile([128, 512], I16, tag="d1")
                dve_t = {"d0": d0_t, "d1": d1_t}

                def diag_mask(dg):
                    # diagonal 128x128 block: zero strictly-upper
                    # (k > q, i.e. free idx c < partition idx p)
                    nc.gpsimd.affine_select(
                        out=dg,
                        in_=dg,
                        pattern=[[1, 128]],
                        compare_op=mybir.AluOpType.is_ge,
                        fill=0.0,
                        base=0,
                        channel_multiplier=-1,
                    )

                def qk_group(gi, kt_t=kt_t, q_t=q_t, pt_t=pt_t, dve_t=dve_t):
                    g = GROUPS[gi]
                    s_t = s_psum.tile([128, SLOT_W], F32, tag="s")

                    def mms(j, local):
                        ext = CHUNK_W[j]
                        # segment matmuls, never crossing a 512-col PSUM bank
                        q0 = 0
                        while q0 < ext:
                            lo = local + q0
                            w = min(512 - (lo % 512), ext - q0)
                            nc.tensor.matmul(
                                s_t[:, lo : lo + w],
                                lhsT=kt_t[:, 128 * j : 128 * (j + 1)],
                                rhs=q_t[:, 128 * j + q0 : 128 * j + q0 + w],
                                start=True,
                                stop=True,
                            )
                            q0 += w

                    for j, local in g["sc"]:
                        mms(j, local)
                    if g["sc"]:
                        lo = g["sc"][0][1]
                        w = sum(CHUNK_W[j] for j, _ in g["sc"])
                        base = SC_OFF[g["sc"][0][0]]
                        nc.scalar.activation(
                            out=pt_t[:, base : base + w],
                            in_=s_t[:, lo : lo + w],
                            func=Exp,
                            scale=SCALE,
                        )
                        for j, _ in g["sc"]:
                            o = SC_OFF[j]
                            diag_mask(pt_t[:, o : o + 128])
                    for j, local in g["dve"]:
                        mms(j, local)
                    if g["dve"]:
                        # dve chunks of one group are contiguous in psum and
                        # in their int16 tile: one tensor_scalar per group
                        j0, lo = g["dve"][0]
                        w = sum(CHUNK_W[j] for j, _ in g["dve"])
                        dt = dve_t[DVE_TILE[j0]]
                        db = DVE_OFF[j0]
                        nc.vector.tensor_scalar(
                            out=dt[:, db : db + w],
                            in0=s_t[:, lo : lo + w],
                            scalar1=EXP_A,
                            scalar2=EXP_B,
                            op0=mybir.AluOpType.mult,
                            op1=mybir.AluOpType.add,
                        )
                        for j, _ in g["dve"]:
                            o = DVE_OFF[j]
                            diag_mask(dt[:, o : o + 128])

                qk_group(0)
                stage[n] = (pt_t, dve_t, vp_t, b, l, qk_group)

            def pv_pack(i_lo, i_hi, st):
                # 3 PV outputs share one PSUM bank; one wide cast per bank
                ppt_t, pdve_t, pvp_t, pb, pl, _ = st

                def lhsT(i, j):
                    if j in DVE_TILE:
                        t = pdve_t[DVE_TILE[j]]
                        co = DVE_OFF[j] + 128 * (i - j)
                        return t[:, co : co + 128].bitcast(BF16)
                    co = SC_OFF[j] + 128 * (i - j)
                    return ppt_t[:, co : co + 128]

                o_t = o_psum.tile([128, i_hi - i_lo, D + 1], F32, tag="o")
                for i in range(i_lo, i_hi):
                    for j in range(i + 1):
                        nc.tensor.matmul(
                            o_t[:, i - i_lo, :],
                            lhsT=lhsT(i, j),
                            rhs=pvp_t[:, j, :],
                            start=(j == 0),
                            stop=(j == i),
                        )
                # unnormalized numerator + denominator column; the
                # softmax divide happens on the host
                nc.vector.tensor_copy(ob_t[:, i_lo:i_hi, :], o_t)
                if n == len(heads):
                    # final head: store each pack as soon as it is cast so
                    # the transfers overlap the kernel-tail drain
                    nc.sync.dma_start(
                        out=out[st[3], st[4], :, i_lo:i_hi],
                        in_=ob_t[:, i_lo:i_hi],
                    )

            if n > 0:
                prev = stage.pop(n - 1)
                ob_t = ob_pool.tile([128, NT, D + 1], BF16, tag="ob")
                pv_pack(0, 3, prev)

            if n < len(heads):
                stage[n][5](1)

            if n > 0:
                pv_pack(3, 6, prev)

            if n < len(heads):
                stage[n][5](2)

            if n > 0:
                pv_pack(6, NT, prev)
                if n < len(heads):
                    nc.sync.dma_start(out=out[prev[3], prev[4]], in_=ob_t)


def _build():
    nc = bacc.Bacc(
        "TRN2", target_bir_lowering=False, debug=False, enable_asserts=False
    )
    qT = nc.dram_tensor("qT", [B, G, D, S], BF16, kind="ExternalInput").ap()
    kT = nc.dram_tensor("kT", [B, D, S], BF16, kind="ExternalInput").ap()
    vp = nc.dram_tensor("vp", [B, 128, NT, D + 1], BF16, kind="ExternalInput").ap()
    out = nc.dram_tensor(
        "out", [B, G, 128, NT, D + 1], BF16, kind="ExternalOutput"
    ).ap()
    with tile.TileContext(nc) as tc:
        _emit(tc, qT, kT, vp, out)
    nc.compile()
    return nc


def get_nc():
    global _NC_CACHE
    if _NC_CACHE is None:
        _NC_CACHE = _build()
    return _NC_CACHE


def make_in_maps(q, k_cache, v_cache, block_table):
    q = np.asarray(q, dtype=np.float32)
    k_cache = np.asarray(k_cache, dtype=np.float32)
    v_cache = np.asarray(v_cache, dtype=np.float32)
    block_table = np.asarray(block_table)

    q_r = q.reshape(B, S, HQ, D)
    in_maps = []
    for c in range(NCORES):
        # [B, G, D, S] query, transposed to d-major
        qT_c = np.ascontiguousarray(
            q_r[:, :, G * c : G * (c + 1), :].transpose(0, 2, 3, 1)
        ).astype(_BF16_NP)
        kT_c = np.empty((B, D, S), dtype=_BF16_NP)
        # [B, 128, NT, D+1]: partition-major V' so device rows are contiguous
        vp_c = np.empty((B, 128, NT, D + 1), dtype=_BF16_NP)
        for b in range(B):
            blocks = block_table[b]  # logical -> physical page ids
            k_seq = k_cache[blocks, :, c, :].reshape(S, D)
            v_seq = v_cache[blocks, :, c, :].reshape(S, D)
            kT_c[b] = k_seq.T.astype(_BF16_NP)
            # token 128*j + p -> vp_c[b, p, j, :]
            vp_c[b, :, :, :D] = (
                v_seq.reshape(NT, 128, D).transpose(1, 0, 2).astype(_BF16_NP)
            )
            vp_c[b, :, :, D] = 1.0
        in_maps.append({"qT": qT_c, "kT": kT_c, "vp": vp_c})
    return in_maps


def assemble_out(results):
    full = np.empty((B, S, HQ, D), dtype=np.float32)
    for c in range(NCORES):
        o = np.asarray(results[c]["out"], dtype=np.float32)  # [B,G,128,NT,D+1]
        # (b, l, p, i, d) -> token 128*i + p
        o = o.transpose(0, 3, 2, 1, 4).reshape(B, S, G, D + 1)
        full[:, :, G * c : G * (c + 1), :] = o[..., :D] / o[..., D:]
    return full.reshape(B * S, HQ * D)


def kernel(q, k_cache, v_cache, block_table):
    nc = get_nc()
    in_maps = make_in_maps(q, k_cache, v_cache, block_table)
    res = run_bass_kernel_spmd(nc, in_maps, core_ids=list(range(NCORES)))
    return assemble_out(res.results)


# revision 12
# speedup vs baseline: 1.1961x; 1.1961x over previous
"""Paged causal GQA prefill attention on 8 TRN2 NeuronCores.

Problem: B=4 seqs x S=1024 tokens, HQ=32 query heads, HK=8 KV heads, D=128,
paged KV cache (16 blocks x 256), causal, softmax scale 1/sqrt(128).

Sharding: tensor-parallel over heads. Core c owns KV head c and the G=4
query heads [4c, 4c+4) for all 4 sequences -> 16 (seq, head) units per core,
perfectly balanced, no collectives (output is disjoint across cores).

Per-unit algorithm (S^T layout, bf16 matmuls, f32 accumulation):
  S^T[k,q] = K^T.T @ Q^T   (lhsT = K^T[d,k] tile, rhs = Q^T[d,q], PSUM f32)
  P^T[k,q] = exp(SCALE * S^T)  split across TWO engines:
     chunks 0,1,2,3,6 (3584 cols): ScalarE ACTIVATE exp (exact)
     chunks 4,5,7 (1024 cols): DVE Schraudolph exp-approx
       bf16(exp(x*SCALE)) ~= bitcast_bf16(int16(x*EXP_A + EXP_B))
  diag blocks: zero k>q half           (GPSIMD affine_select)
  O[q, 0:129] = sum_j P^T_j.T @ [V_j | 1]  (PSUM accumulate over k chunks;
               col 128 is the softmax denominator, no separate reduction)
  host divides numerator by denominator column.

Why the split: ScalarE ACTIVATE costs (N+352)/1.2 ns; all-exp-on-ScalarE is
74us vs ~66us of PE matmul -> ScalarE was the trace bottleneck (82% busy).
Moving 1536 cols to DVE rebalances: ScalarE ~55us, DVE ~58us (incl. the
o-PSUM casts), PE ~66-70us critical. The Schraudolph common-mode error
cancels in the softmax normalization (denominator sums the same approx
values); the residual ~2% rms per-weight ripple lands well under the 2e-2
gate (measured).

Score PSUM groups (3-bank tiles, 2-slot rotation as in the 91us baseline —
a 4-tile rotation measured slower from tighter producer-consumer slack).
ScalarE and DVE slices of one group tile never share a PSUM bank
(concurrent ScE/DVE reads of one bank are not allowed):
  G0 = [c0 sc 0:1024            | c4 dve 1024:1536]      (banks 0-1 | 2)
  G1 = [c2 sc 0:768 c6 768:1024 | c5,c7 dve 1024:1536]   (banks 0-1 | 2)
  G2 = [c1 sc 0:896 c3 896:1536]
DVE queue order per head: TS(c4), cast p0, TS(c5c7), cast p1, cast p2 —
every cast is enqueued immediately after its PV pack so o-PSUM recycling
never waits behind exp work.

Output DRAM layout is [B, G, 128, NT, D+1] (partition-major, identical to
the SBUF ob tile) so the store is one fully-contiguous 2064B-per-partition
DMA per head; the host re-permutes. DMA triggers are merged (the SP queue
was 70% busy on 661ns-per-call descriptor generation in the baseline).
"""

import numpy as np
import ml_dtypes
import math as _math
from contextlib import ExitStack

import concourse.bass as bass
import concourse.tile as tile
from concourse import bacc, mybir
from concourse.bass_utils import run_bass_kernel_spmd

B, S, HQ, HK, D = 4, 1024, 32, 8, 128
BS = 256
G = HQ // HK            # 4 query heads per KV head
NCORES = 8
NT = S // 128           # 8 key chunks / query tiles of 128
SCALE = 1.0 / float(np.sqrt(D))

BF16 = mybir.dt.bfloat16
F32 = mybir.dt.float32
I16 = mybir.dt.int16
_BF16_NP = ml_dtypes.bfloat16

EXP_A = SCALE * 128.0 / _math.log(2.0)  # fold softmax scale into the fma
EXP_B = 128.0 * 127.0 - 6.0             # bias, c=6 calibrated for min rms

# chunk j covers keys [128j, 128(j+1)) and queries q in [128j, S)
CHUNK_W = {j: S - 128 * j for j in range(NT)}

# ScalarE chunks packed in the pt tile (bf16 SBUF), group-major
SC_OFF = {0: 0, 2: 1024, 6: 1792, 1: 2048, 3: 2944}
PT_COLS = 3584
# DVE chunks: d0 tile holds chunk 4 (512), d1 tile holds chunks 5+7 (512)
DVE_TILE = {4: "d0", 5: "d1", 7: "d1"}
DVE_OFF = {4: 0, 5: 0, 7: 384}

# score psum groups: ("sc"/"dve" chunk lists with psum col offsets).
# DVE slices start on a PSUM bank boundary (col 1024 = bank 2 of the
# 3-bank group tile) so ScalarE and DVE never read the same bank.
# Chunk 0 stays on ScalarE: it is the first accumulation step of every
# PV output tile, and routing it through DVE measured -11us.
GROUPS = [
    {"sc": [(0, 0)], "dve": [(4, 1024)]},
    {"sc": [(2, 0), (6, 768)], "dve": [(5, 1024), (7, 1408)]},
    {"sc": [(1, 0), (3, 896)], "dve": []},
]
SLOT_W = 1536

_NC_CACHE = None


def _emit(tc, qT, kT, vp, out):
    nc = tc.nc
    Exp = mybir.ActivationFunctionType.Exp

    with ExitStack() as ctx:
        kv_pool = ctx.enter_context(tc.tile_pool(name="kv", bufs=3))
        q_pool = ctx.enter_context(tc.tile_pool(name="q", bufs=4))
        pt_pool = ctx.enter_context(tc.tile_pool(name="pt", bufs=3))
        dve_pool = ctx.enter_context(tc.tile_pool(name="dve", bufs=3))
        s_psum = ctx.enter_context(tc.tile_pool(name="s_psum", bufs=2, space="PSUM"))
        o_psum = ctx.enter_context(tc.tile_pool(name="o_psum", bufs=2, space="PSUM"))
        ob_pool = ctx.enter_context(tc.tile_pool(name="ob", bufs=6))
        singles = ctx.enter_context(tc.tile_pool(name="singles", bufs=1))

        # trigger the exp ACT_TABLE_LOAD (~2.7us) during the initial DMAs
        warm = singles.tile([1, 1], F32)
        nc.vector.memset(warm, 0.0)
        nc.scalar.activation(out=warm, in_=warm, func=Exp)

        heads = [(b, l) for b in range(B) for l in range(G)]
        stage = {}
        kv_cur = None

        def load_kv(bb):
            kt_t = kv_pool.tile([D, S], BF16, tag="kt")
            nc.sync.dma_start(out=kt_t[:, :128], in_=kT[bb][:, :128])
            nc.sync.dma_start(out=kt_t[:, 128:], in_=kT[bb][:, 128:])
            vp_t = kv_pool.tile([128, NT, D + 1], BF16, tag="vp")
            nc.sync.dma_start(out=vp_t, in_=vp[bb])
            return kt_t, vp_t

        # Software pipeline staggered by one head: PE runs QK^T(n) while
        # ScalarE/DVE exp head n-1..n scores; PV(n-1) P^T is ready by then.
        # q tiles are prefetched ONE HEAD AHEAD: the in-order PE queue
        # head-of-line blocks on the q transfer if it is triggered in the
        # same iteration that consumes it.
        q_tiles = {}
        for n in range(len(heads) + 1):
            if n + 1 < len(heads) and n >= 1:
                nb, nl = heads[n + 1]
                qn = q_pool.tile([D, S], BF16, tag="q")
                nc.sync.dma_start(out=qn, in_=qT[nb, nl])
                q_tiles[n + 1] = qn
            if n < len(heads):
                b, l = heads[n]
                if n == 0:
                    kt0 = kv_pool.tile([D, S], BF16, tag="kt")
                    nc.sync.dma_start(out=kt0[:, :128], in_=kT[0][:, :128])
                    q_t = q_pool.tile([D, S], BF16, tag="q")
                    nc.sync.dma_start(out=q_t[:, :512], in_=qT[b, l][:, :512])
                    nc.sync.dma_start(out=q_t[:, 512:], in_=qT[b, l][:, 512:])
                    nc.sync.dma_start(out=kt0[:, 128:], in_=kT[0][:, 128:])
                    vp0 = kv_pool.tile([128, NT, D + 1], BF16, tag="vp")
                    nc.sync.dma_start(out=vp0, in_=vp[0])
                    kv_next = (kt0, vp0)
                    q1 = q_pool.tile([D, S], BF16, tag="q")
                    nc.sync.dma_start(out=q1, in_=qT[heads[1][0], heads[1][1]])
                    q_tiles[1] = q1
                else:
                    q_t = q_tiles.pop(n)
                if l == 0:
                    kv_cur, kv_next = kv_next, None
                if l == G - 1 and b + 1 < B:
                    # prefetch the next sequence's K/V one head early
                    kv_next = load_kv(b + 1)
                kt_t, vp_t = kv_cur

                pt_t = pt_pool.tile([128, PT_COLS], BF16, tag="pt")
                d0_t = dve_pool.tile([128, 512], I16, tag="d0")
                d1_t = dve_pool.tile([128, 512], I16, tag="d1")
                dve_t = {"d0": d0_t, "d1": d1_t}

                def diag_mask(dg):
                    # diagonal 128x128 block: zero strictly-upper
                    # (k > q, i.e. free idx c < partition idx p)
                    nc.gpsimd.affine_select(
                        out=dg,
                        in_=dg,
                        pattern=[[1, 128]],
                        compare_op=mybir.AluOpType.is_ge,
                        fill=0.0,
                        base=0,
                        channel_multiplier=-1,
                    )

                def qk_group(gi, kt_t=kt_t, q_t=q_t, pt_t=pt_t, dve_t=dve_t):
                    g = GROUPS[gi]
                    s_t = s_psum.tile([128, SLOT_W], F32, tag="s")

                    def mms(j, local):
                        ext = CHUNK_W[j]
                        # segment matmuls, never crossing a 512-col PSUM bank
                        q0 = 0
                        while q0 < ext:
                            lo = local + q0
                            w = min(512 - (lo % 512), ext - q0)
                            nc.tensor.matmul(
                                s_t[:, lo : lo + w],
                                lhsT=kt_t[:, 128 * j : 128 * (j + 1)],
                                rhs=q_t[:, 128 * j + q0 : 128 * j + q0 + w],
                                start=True,
                                stop=True,
                            )
                            q0 += w

                    for j, local in g["sc"]:
                        mms(j, local)
                    if g["sc"]:
                        lo = g["sc"][0][1]
                        w = sum(CHUNK_W[j] for j, _ in g["sc"])
                        base = SC_OFF[g["sc"][0][0]]
                        nc.scalar.activation(
                            out=pt_t[:, base : base + w],
                            in_=s_t[:, lo : lo + w],
                            func=Exp,
                            scale=SCALE,
                        )
                        for j, _ in g["sc"]:
                            o = SC_OFF[j]
                            diag_mask(pt_t[:, o : o + 128])
                    for j, local in g["dve"]:
                        mms(j, local)
                    if g["dve"]:
                        # dve chunks of one group are contiguous in psum and
                        # in their int16 tile: one tensor_scalar per group
                        j0, lo = g["dve"][0]
                        w = sum(CHUNK_W[j] for j, _ in g["dve"])
                        dt = dve_t[DVE_TILE[j0]]
                        db = DVE_OFF[j0]
                        nc.vector.tensor_scalar(
                            out=dt[:, db : db + w],
                            in0=s_t[:, lo : lo + w],
                            scalar1=EXP_A,
                            scalar2=EXP_B,
                            op0=mybir.AluOpType.mult,
                            op1=mybir.AluOpType.add,
                        )
                        for j, _ in g["dve"]:
                            o = DVE_OFF[j]
                            diag_mask(dt[:, o : o + 128])

                qk_group(0)
                stage[n] = (pt_t, dve_t, vp_t, b, l, qk_group)

            def pv_pack(i_lo, i_hi, st):
                # 3 PV outputs share one PSUM bank; one wide cast per bank
                ppt_t, pdve_t, pvp_t, pb, pl, _ = st

                def lhsT(i, j):
                    if j in DVE_TILE:
                        t = pdve_t[DVE_TILE[j]]
                        co = DVE_OFF[j] + 128 * (i - j)
                        return t[:, co : co + 128].bitcast(BF16)
                    co = SC_OFF[j] + 128 * (i - j)
                    return ppt_t[:, co : co + 128]

                o_t = o_psum.tile([128, i_hi - i_lo, D + 1], F32, tag="o")
                for i in range(i_lo, i_hi):
                    for j in range(i + 1):
                        nc.tensor.matmul(
                            o_t[:, i - i_lo, :],
                            lhsT=lhsT(i, j),
                            rhs=pvp_t[:, j, :],
                            start=(j == 0),
                            stop=(j == i),
                        )
                # unnormalized numerator + denominator column; the
                # softmax divide happens on the host
                nc.vector.tensor_copy(ob_t[:, i_lo:i_hi, :], o_t)
                if n == len(heads):
                    # final head: store each pack as soon as it is cast so
                    # the transfers overlap the kernel-tail drain
                    nc.sync.dma_start(
                        out=out[st[3], st[4], :, i_lo:i_hi],
                        in_=ob_t[:, i_lo:i_hi],
                    )

            if n > 0:
                prev = stage.pop(n - 1)
                ob_t = ob_pool.tile([128, NT, D + 1], BF16, tag="ob")
                pv_pack(0, 3, prev)

            if n < len(heads):
                stage[n][5](1)

            if n > 0:
                pv_pack(3, 6, prev)

            if n < len(heads):
                stage[n][5](2)

            if n > 0:
                pv_pack(6, NT, prev)
                if n < len(heads):
                    nc.sync.dma_start(out=out[prev[3], prev[4]], in_=ob_t)


def _build():
    nc = bacc.Bacc(
        "TRN2", target_bir_lowering=False, debug=False, enable_asserts=False
    )
    qT = nc.dram_tensor("qT", [B, G, D, S], BF16, kind="ExternalInput").ap()
    kT = nc.dram_tensor("kT", [B, D, S], BF16, kind="ExternalInput").ap()
    vp = nc.dram_tensor("vp", [B, 128, NT, D + 1], BF16, kind="ExternalInput").ap()
    out = nc.dram_tensor(
        "out", [B, G, 128, NT, D + 1], BF16, kind="ExternalOutput"
    ).ap()
    with tile.TileContext(nc) as tc:
        _emit(tc, qT, kT, vp, out)
    nc.compile()
    return nc


def get_nc():
    global _NC_CACHE
    if _NC_CACHE is None:
        _NC_CACHE = _build()
    return _NC_CACHE


def make_in_maps(q, k_cache, v_cache, block_table):
    q = np.asarray(q, dtype=np.float32)
    k_cache = np.asarray(k_cache, dtype=np.float32)
    v_cache = np.asarray(v_cache, dtype=np.float32)
    block_table = np.asarray(block_table)

    q_r = q.reshape(B, S, HQ, D)
    in_maps = []
    for c in range(NCORES):
        # [B, G, D, S] query, transposed to d-major
        qT_c = np.ascontiguousarray(
            q_r[:, :, G * c : G * (c + 1), :].transpose(0, 2, 3, 1)
        ).astype(_BF16_NP)
        kT_c = np.empty((B, D, S), dtype=_BF16_NP)
        # [B, 128, NT, D+1]: partition-major V' so device rows are contiguous
        vp_c = np.empty((B, 128, NT, D + 1), dtype=_BF16_NP)
        for b in range(B):
            blocks = block_table[b]  # logical -> physical page ids
            k_seq = k_cache[blocks, :, c, :].reshape(S, D)
            v_seq = v_cache[blocks, :, c, :].reshape(S, D)
            kT_c[b] = k_seq.T.astype(_BF16_NP)
            # token 128*j + p -> vp_c[b, p, j, :]
            vp_c[b, :, :, :D] = (
                v_seq.reshape(NT, 128, D).transpose(1, 0, 2).astype(_BF16_NP)
            )
            vp_c[b, :, :, D] = 1.0
        in_maps.append({"qT": qT_c, "kT": kT_c, "vp": vp_c})
    return in_maps


def assemble_out(results):
    full = np.empty((B, S, HQ, D), dtype=np.float32)
    for c in range(NCORES):
        o = np.asarray(results[c]["out"], dtype=np.float32)  # [B,G,128,NT,D+1]
        # (b, l, p, i, d) -> token 128*i + p
        o = o.transpose(0, 3, 2, 1, 4).reshape(B, S, G, D + 1)
        full[:, :, G * c : G * (c + 1), :] = o[..., :D] / o[..., D:]
    return full.reshape(B * S, HQ * D)


def kernel(q, k_cache, v_cache, block_table):
    nc = get_nc()
    in_maps = make_in_maps(q, k_cache, v_cache, block_table)
    res = run_bass_kernel_spmd(nc, in_maps, core_ids=list(range(NCORES)))
    return assemble_out(res.results)
